# revision 22
# baseline (speedup 1.0000x reference)
"""Trainium2 Bass kernel for the distributed CLIP-style contrastive loss.

loss = 0.5 * ( mean_i( LSE_row(i) - diag(i) ) + mean_j( LSE_col(j) - diag(j) ) )
with logits = tau * ftir @ raman.T, tau = min(exp(log_tau), 100), B=4096, D=512.

Sharding: rows of the [B, B] logits matrix are split across 8 cores (512 rows
each).  Each core computes BOTH its row-slab of logits (ftir_shard @ raman.T)
and its row-slab of logits.T (raman_shard @ ftir.T), so the column-softmax is
just a second row-softmax and no collectives are needed.  Row log-sum-exp is
computed with an exact two-level scheme: per 1024-wide block the VectorE takes
the block max straight out of PSUM (negated, as the exp bias), the ScalarE
computes exp(x - m_b) with a fused free-dim accumulation (accum_out), and the
host combines block stats exactly: LSE = M + log(sum_b s_b * exp(m_b - M)).

Matmuls run in fp8 (e4m3, TRN flavor: max +-240) with perf_mode=DoubleRow,
which packs two fp8 weights per PE cell (virtual 128x256 array): each matmul
contracts K=256 in one instruction at ~2x the bf16 MAC rate.  Inputs are
laid out as [128, 2, n] tiles where dim1 selects the K-half (k, k+128).

Each core returns raw per-block stats (negm/sums, [128, 32]) and the diagonal
dot products ([1, 512]); the host does the exact two-level LSE combine and the
final scalar reduction in float64.
"""

import sys

import numpy as np

for _p in ("/opt/trn_rl_repo", "/root/.axon_site/_ro/trn_rl_repo"):
    if _p not in sys.path:
        sys.path.append(_p)

from contextlib import ExitStack

import concourse.bacc as bacc
import concourse.tile as tile
from concourse import mybir
from concourse.bass_utils import run_bass_kernel_spmd

B = 4096
D = 512
NCORES = 8
SH = B // NCORES  # 512 rows per core
P = 128
KS = D // 256  # 2 K-super-chunks of 256 (DoubleRow packs 2x128)
MT = SH // P  # 4 m-tiles of 128 rows
BLK = 1024  # PSUM stats-block width
NB = B // BLK  # 4 blocks per row
SUB = 512  # matmul N per instruction
CHW = 2048  # DMA chunk width for the full tensors

DT_IN = mybir.dt.float8e4
DT_SCR = mybir.dt.bfloat16  # exp scratch output dtype (value is discarded)

F32 = mybir.dt.float32
AX = mybir.AxisListType
ALU = mybir.AluOpType
ACTF = mybir.ActivationFunctionType
PM_DR = mybir.MatmulPerfMode.DoubleRow

# toggled by test harness for profiling
PROFILE = False
LAST_RESULTS = None

_prog_cache = {}


def _build_program(dt_in):
    nc = bacc.Bacc(
        "TRN2",
        target_bir_lowering=False,
        debug=False,
        enable_partition_id=False,
        enable_asserts=False,
    )

    # host pre-permutes every tensor to [p, ...] partition-major so each DMA
    # below is a single contiguous-per-partition 2D descriptor at full HBM
    # bandwidth (a 4D gather of 512B segments measured only ~190GB/s).
    # ats/bts: [p, (s h) e];  atf/btf: [p, ch (s h) e_chunk] chunk-major.
    ats = nc.dram_tensor("ats", [P, 2 * KS * SH], dt_in, kind="ExternalInput").ap()
    bts = nc.dram_tensor("bts", [P, 2 * KS * SH], dt_in, kind="ExternalInput").ap()
    atf = nc.dram_tensor("atf", [P, 2 * KS * B], dt_in, kind="ExternalInput").ap()
    btf = nc.dram_tensor("btf", [P, 2 * KS * B], dt_in, kind="ExternalInput").ap()
    # negm/sums interleaved per stats column ([P, col, 2]) so each output DMA
    # region is contiguous and the tail descriptor only covers the last blocks
    stats_out = nc.dram_tensor(
        "stats", [P, 2 * 2 * MT * NB], F32, kind="ExternalOutput"
    ).ap()

    with ExitStack() as ctx:
        tc = ctx.enter_context(tile.TileContext(nc))
        inp = ctx.enter_context(tc.tile_pool(name="inp", bufs=1))
        psum = ctx.enter_context(tc.tile_pool(name="psum", bufs=4, space="PSUM"))
        scr = ctx.enter_context(tc.tile_pool(name="scr", bufs=3))

        # ---- PE warm-up: a couple of dummy matmuls while input DMAs stream
        # in, so the PE pipeline/pstate is past the cold state when the first
        # real matmul issues.  Stats (not PE) are the critical path, so a
        # short warm-up that lets real blocks start ~6us earlier beats a long
        # one that reaches full clock before any real work.  memset on the
        # (otherwise idle) GpSimd so VE isn't serialized into the warm path.
        warm_sb = inp.tile([P, SUB], dt_in, tag="warm_sb")
        nc.gpsimd.memset(warm_sb, 0.0)
        # dummy exp primes the ACT Exp table during the DMA-bound head —
        # otherwise the lazy ACT_TABLE_LOAD (1.28us) lands right before the
        # first real exp and delays the first PSUM release.
        warm_act = inp.tile([P, 1], F32, tag="warm_act")
        nc.scalar.activation(warm_act, warm_sb[:, 0:1], ACTF.Exp)
        warm_ps = psum.tile([P, BLK], F32, tag="ps")
        for _ in range(2):
            nc.tensor.matmul(
                warm_ps[:, :SUB], lhsT=warm_sb[:, :P], rhs=warm_sb, start=True, stop=True
            )

        # ---- persistent input tiles.  DoubleRow layout: [128, 4, n] where
        # dim1 = (super s)*2 + (K-half h); a matmul for super s slices
        # [:, 2s:2s+2, :].  Each region loads with ONE DMA descriptor (the
        # sequencer pays ~610ns of dispatch per dma_start, so descriptor
        # count — not bandwidth — dominated the old head). ----
        a_sh = inp.tile([P, 2 * KS, SH], dt_in, tag="ash")
        b_sh = inp.tile([P, 2 * KS, SH], dt_in, tag="bsh")

        # full tensors as separate chunk tiles for fine-grained DMA deps.
        # b gets narrow leading chunks so the very first psum tile's inputs
        # land quickly; the bulk arrives in 2048-wide chunks.
        B_EDGES = [0, 1024, 2048, 3072, 4096]
        A_EDGES = [0, 2048, 4096]

        def chunked_alloc(name, edges):
            return [
                inp.tile(
                    [P, 2 * KS, edges[ch + 1] - edges[ch]],
                    dt_in,
                    tag=f"{name}_{ch}",
                    name=f"{name}_{ch}",
                )
                for ch in range(len(edges) - 1)
            ]

        b_f = chunked_alloc("bf", B_EDGES)
        a_f = chunked_alloc("af", A_EDGES)

        def chunk_of(edges, n0):
            for ch in range(len(edges) - 1):
                if n0 < edges[ch + 1]:
                    return ch, n0 - edges[ch]
            raise AssertionError

        G = 2 * KS  # (s h) group count

        # Head-critical inputs issue from GpSimd's DGE: its sequencer starts
        # ~1us before the Sync sequencer's tile-context, and splitting the
        # descriptor dispatch across two sequencers removes the serial
        # ~720ns-per-descriptor bottleneck in front of the first block.
        nc.gpsimd.dma_start(out=a_sh, in_=ats)
        nc.gpsimd.dma_start(out=b_f[0], in_=btf[:, G * B_EDGES[0] : G * B_EDGES[1]])
        nc.gpsimd.dma_start(out=b_f[1], in_=btf[:, G * B_EDGES[1] : G * B_EDGES[2]])
        nc.sync.dma_start(out=b_sh, in_=bts)
        for ch in range(2, len(B_EDGES) - 1):
            nc.sync.dma_start(
                out=b_f[ch], in_=btf[:, G * B_EDGES[ch] : G * B_EDGES[ch + 1]]
            )
        for ch in range(len(A_EDGES) - 1):
            nc.sync.dma_start(
                out=a_f[ch], in_=atf[:, G * A_EDGES[ch] : G * A_EDGES[ch + 1]]
            )

        # raw per-block stats, interleaved [P, col, (negm|sum)] in t-major
        # column order so later blocks occupy later columns; the exact
        # two-level LSE combine happens on the host (removes Ln/table-load
        # and all small fixup ops from the tail).  The diagonal dot products
        # are also computed host-side from the same quantized fp8 inputs,
        # freeing PSUM bank 8 and the GpSimd/ones path.
        stats_all = inp.tile([P, 2 * 2 * MT * NB], F32, tag="stats_all")

        # ---- main two passes ----
        HC = MT * NB  # stats columns per pass
        for L in range(2):
            lhs = a_sh if L == 0 else b_sh
            rhs_t = b_f if L == 0 else a_f  # noqa
            edges = B_EDGES if L == 0 else A_EDGES
            # t outer / m inner: during the DMA ramp all MT psum tiles of a
            # given t consume the SAME 1024-wide rhs slice, so the PE extracts
            # 4x more work per DMA'd byte and never outruns HBM.
            for t in range(NB):
                for m in range(MT):
                    scol = 2 * (L * HC + t * MT + m)
                    ps = psum.tile([P, BLK], F32, tag="ps")
                    for j in range(BLK // SUB):
                        n0 = t * BLK + j * SUB
                        chi, off = chunk_of(edges, n0)
                        for s in range(KS):
                            nc.tensor.matmul(
                                ps[:, j * SUB : (j + 1) * SUB],
                                lhsT=lhs[:, 2 * s : 2 * s + 2, m * P : (m + 1) * P],
                                rhs=rhs_t[chi][:, 2 * s : 2 * s + 2, off : off + SUB],
                                start=(s == 0),
                                stop=(s == KS - 1),
                                perf_mode=PM_DR,
                            )
                    # block stats straight from PSUM
                    nc.vector.reduce_max(
                        out=stats_all[:, scol : scol + 1],
                        in_=ps,
                        axis=AX.X,
                        negate=True,
                    )
                    sc = scr.tile([P, BLK], DT_SCR, tag="escr")
                    nc.scalar.activation(
                        sc,
                        ps,
                        ACTF.Exp,
                        bias=stats_all[:, scol : scol + 1],
                        accum_out=stats_all[:, scol + 1 : scol + 2],
                    )
                # stream completed stats out eagerly so the final (blocking)
                # descriptor in the tail only covers the last few blocks
                if (L, t) == (0, NB - 1):
                    nc.gpsimd.dma_start(
                        out=stats_out[:, : 2 * HC], in_=stats_all[:, : 2 * HC]
                    )
                elif (L, t) == (1, NB - 2):
                    c0, c1 = 2 * HC, 2 * (2 * HC - MT)
                    nc.gpsimd.dma_start(
                        out=stats_out[:, c0:c1], in_=stats_all[:, c0:c1]
                    )
                elif (L, t) == (1, NB - 1):
                    c0 = 2 * (2 * HC - MT)
                    nc.gpsimd.dma_start(
                        out=stats_out[:, c0:], in_=stats_all[:, c0:]
                    )

    nc.compile()
    return nc


def _get_program(dt_in):
    key = str(dt_in)
    if key not in _prog_cache:
        _prog_cache[key] = _build_program(dt_in)
    return _prog_cache[key]


def kernel(out_ftir, out_raman, labels=None, log_tau=None, **_unused):
    global LAST_RESULTS
    out_ftir = np.asarray(out_ftir, dtype=np.float32)
    out_raman = np.asarray(out_raman, dtype=np.float32)
    tau = float(np.minimum(np.exp(np.float64(np.asarray(log_tau))), 100.0))

    np_dt = mybir.dt.np(DT_IN)
    aT = np.ascontiguousarray(
        np.clip((out_ftir * np.float32(tau)).T, -240.0, 240.0)
    ).astype(np_dt)
    bT = np.ascontiguousarray(np.clip(out_raman.T, -240.0, 240.0)).astype(np_dt)

    # diagonal logits host-side from the same quantized values the device
    # matmuls consume: diag[i] = sum_d aT[d,i] * bT[d,i]
    diag_full = np.einsum(
        "di,di->i",
        aT.astype(np.float32),
        bT.astype(np.float32),
        dtype=np.float64,
    )

    def pack_sh(xT):
        # [D, n] -> [p, (s h) e]: row d = s*256 + h*128 + p
        n = xT.shape[1]
        return np.ascontiguousarray(
            xT.reshape(KS, 2, P, n).transpose(2, 0, 1, 3).reshape(P, 2 * KS * n)
        )

    def pack_chunked(xT, edges):
        # [D, B] -> [p, ch (s h) e_chunk]
        x4 = xT.reshape(KS, 2, P, B)
        parts = [
            x4[:, :, :, edges[ch] : edges[ch + 1]].transpose(2, 0, 1, 3).reshape(P, -1)
            for ch in range(len(edges) - 1)
        ]
        return np.ascontiguousarray(np.concatenate(parts, axis=1))

    B_EDGES = [0, 1024, 2048, 3072, 4096]
    A_EDGES = [0, 2048, 4096]
    atf_h = pack_chunked(aT, A_EDGES)
    btf_h = pack_chunked(bT, B_EDGES)

    in_maps = []
    for c in range(NCORES):
        sl = slice(c * SH, (c + 1) * SH)
        in_maps.append(
            {
                "ats": pack_sh(aT[:, sl]),
                "bts": pack_sh(bT[:, sl]),
                "atf": atf_h,
                "btf": btf_h,
            }
        )

    nc = _get_program(DT_IN)
    res = run_bass_kernel_spmd(
        nc, in_maps, core_ids=list(range(NCORES)), trace=PROFILE
    )
    LAST_RESULTS = res

    s_lse = 0.0
    for r in res.results:
        # exact two-level LSE combine (float64):
        # LSE = M + log(sum_b s_b * exp(m_b - M)),  m_b = -negm
        # stats layout: [P, L, t, m, (negm|sum)], block index t combines
        st = r["stats"].astype(np.float64).reshape(P, 2, NB, MT, 2)
        mb = -st[..., 0]
        sb = st[..., 1]
        M = mb.max(axis=2, keepdims=True)
        lse = M[:, :, 0, :] + np.log((sb * np.exp(mb - M)).sum(axis=2))
        s_lse += float(lse.sum())
    s_diag = float(diag_full.sum())
    loss = (s_lse - 2.0 * s_diag) / (2.0 * B)
    return np.array(loss, dtype=np.float32)


# revision 25
# speedup vs baseline: 1.1014x; 1.1014x over previous
"""Trainium2 Bass kernel for the distributed CLIP-style contrastive loss.

loss = 0.5 * ( mean_i( LSE_row(i) - diag(i) ) + mean_j( LSE_col(j) - diag(j) ) )
with logits = tau * ftir @ raman.T, tau = min(exp(log_tau), 100), B=4096, D=512.

Sharding: rows of the [B, B] logits matrix are split across 8 cores (512 rows
each).  Each core computes BOTH its row-slab of logits (ftir_shard @ raman.T)
and its row-slab of logits.T (raman_shard @ ftir.T), so the column-softmax is
just a second row-softmax and no collectives are needed.  Row log-sum-exp is
computed with an exact two-level scheme: per 1024-wide block the VectorE takes
the block max straight out of PSUM (negated, as the exp bias), the ScalarE
computes exp(x - m_b) with a fused free-dim accumulation (accum_out), and the
host combines block stats exactly: LSE = M + log(sum_b s_b * exp(m_b - M)).

Matmuls run in fp8 (e4m3, TRN flavor: max +-240) with perf_mode=DoubleRow,
which packs two fp8 weights per PE cell (virtual 128x256 array): each matmul
contracts K=256 in one instruction at ~2x the bf16 MAC rate.  Inputs are
laid out as [128, 2, n] tiles where dim1 selects the K-half (k, k+128).

Each core returns raw per-block stats (negm/sums, [128, 32]) and the diagonal
dot products ([1, 512]); the host does the exact two-level LSE combine and the
final scalar reduction in float64.
"""

import sys

import numpy as np

for _p in ("/opt/trn_rl_repo", "/root/.axon_site/_ro/trn_rl_repo"):
    if _p not in sys.path:
        sys.path.append(_p)

from contextlib import ExitStack

import concourse.bacc as bacc
import concourse.tile as tile
from concourse import mybir
from concourse.bass_utils import run_bass_kernel_spmd

B = 4096
D = 512
NCORES = 8
SH = B // NCORES  # 512 rows per core
P = 128
KS = D // 256  # 2 K-super-chunks of 256 (DoubleRow packs 2x128)
MT = SH // P  # 4 m-tiles of 128 rows
BLK = 1024  # PSUM stats-block width
NB = B // BLK  # 4 blocks per row
SUB = 512  # matmul N per instruction
CHW = 2048  # DMA chunk width for the full tensors

DT_IN = mybir.dt.float8e4
DT_SCR = mybir.dt.bfloat16  # exp scratch output dtype (value is discarded)

F32 = mybir.dt.float32
AX = mybir.AxisListType
ALU = mybir.AluOpType
ACTF = mybir.ActivationFunctionType
PM_DR = mybir.MatmulPerfMode.DoubleRow

# toggled by test harness for profiling
PROFILE = False
LAST_RESULTS = None

_prog_cache = {}


def _build_program(dt_in):
    nc = bacc.Bacc(
        "TRN2",
        target_bir_lowering=False,
        debug=False,
        enable_partition_id=False,
        enable_asserts=False,
    )

    # host pre-permutes every tensor to [p, ...] partition-major so each DMA
    # below is a single contiguous-per-partition 2D descriptor at full HBM
    # bandwidth (a 4D gather of 512B segments measured only ~190GB/s).
    # ats/bts: [p, (s h) e];  atf/btf: [p, ch (s h) e_chunk] chunk-major.
    ats = nc.dram_tensor("ats", [P, 2 * KS * SH], dt_in, kind="ExternalInput").ap()
    bts = nc.dram_tensor("bts", [P, 2 * KS * SH], dt_in, kind="ExternalInput").ap()
    atf = nc.dram_tensor("atf", [P, 2 * KS * B], dt_in, kind="ExternalInput").ap()
    btf = nc.dram_tensor("btf", [P, 2 * KS * B], dt_in, kind="ExternalInput").ap()
    # negm/sums interleaved per stats column ([P, col, 2]) so each output DMA
    # region is contiguous and the tail descriptor only covers the last blocks
    stats_out = nc.dram_tensor(
        "stats", [P, 2 * 2 * MT * NB], F32, kind="ExternalOutput"
    ).ap()

    with ExitStack() as ctx:
        tc = ctx.enter_context(tile.TileContext(nc))
        inp = ctx.enter_context(tc.tile_pool(name="inp", bufs=1))
        psum = ctx.enter_context(tc.tile_pool(name="psum", bufs=4, space="PSUM"))
        scr = ctx.enter_context(tc.tile_pool(name="scr", bufs=3))

        # ---- PE warm-up: a couple of dummy matmuls while input DMAs stream
        # in, so the PE pipeline/pstate is past the cold state when the first
        # real matmul issues.  Stats (not PE) are the critical path, so a
        # short warm-up that lets real blocks start ~6us earlier beats a long
        # one that reaches full clock before any real work.  memset on the
        # (otherwise idle) GpSimd so VE isn't serialized into the warm path.
        warm_sb = inp.tile([P, SUB], dt_in, tag="warm_sb")
        nc.gpsimd.memset(warm_sb, 0.0)
        # dummy exp primes the ACT Exp table during the DMA-bound head —
        # otherwise the lazy ACT_TABLE_LOAD (1.28us) lands right before the
        # first real exp and delays the first PSUM release.
        warm_act = inp.tile([P, 1], F32, tag="warm_act")
        nc.scalar.activation(warm_act, warm_sb[:, 0:1], ACTF.Exp)
        warm_ps = psum.tile([P, BLK], F32, tag="ps")
        for _ in range(4):
            nc.tensor.matmul(
                warm_ps[:, :SUB], lhsT=warm_sb[:, :P], rhs=warm_sb, start=True, stop=True
            )

        # ---- persistent input tiles.  DoubleRow layout: [128, 4, n] where
        # dim1 = (super s)*2 + (K-half h); a matmul for super s slices
        # [:, 2s:2s+2, :].  Each region loads with ONE DMA descriptor (the
        # sequencer pays ~610ns of dispatch per dma_start, so descriptor
        # count — not bandwidth — dominated the old head). ----
        a_sh = inp.tile([P, 2 * KS, SH], dt_in, tag="ash")
        b_sh = inp.tile([P, 2 * KS, SH], dt_in, tag="bsh")

        # full tensors as separate chunk tiles for fine-grained DMA deps.
        # b gets narrow leading chunks so the very first psum tile's inputs
        # land quickly; the bulk arrives in 2048-wide chunks.
        B_EDGES = [0, 512, 1024, 2048, 3072, 4096]
        A_EDGES = [0, 2048, 4096]

        def chunked_alloc(name, edges):
            return [
                inp.tile(
                    [P, 2 * KS, edges[ch + 1] - edges[ch]],
                    dt_in,
                    tag=f"{name}_{ch}",
                    name=f"{name}_{ch}",
                )
                for ch in range(len(edges) - 1)
            ]

        b_f = chunked_alloc("bf", B_EDGES)
        a_f = chunked_alloc("af", A_EDGES)

        def chunk_of(edges, n0):
            for ch in range(len(edges) - 1):
                if n0 < edges[ch + 1]:
                    return ch, n0 - edges[ch]
            raise AssertionError

        G = 2 * KS  # (s h) group count

        # single ordered HWDGE queue: strict consumption order so the head
        # chunks get full HBM bandwidth, with the first block's inputs first.
        # (GpSimd's software DGE measured ~5us SLOWER moving the head inputs
        # — keep inputs on the Sync HWDGE.)
        nc.sync.dma_start(out=a_sh, in_=ats)
        for ch in range(2):
            nc.sync.dma_start(
                out=b_f[ch], in_=btf[:, G * B_EDGES[ch] : G * B_EDGES[ch + 1]]
            )
        nc.sync.dma_start(out=b_sh, in_=bts)
        for ch in range(2, len(B_EDGES) - 1):
            nc.sync.dma_start(
                out=b_f[ch], in_=btf[:, G * B_EDGES[ch] : G * B_EDGES[ch + 1]]
            )
        for ch in range(len(A_EDGES) - 1):
            nc.sync.dma_start(
                out=a_f[ch], in_=atf[:, G * A_EDGES[ch] : G * A_EDGES[ch + 1]]
            )

        # raw per-block stats, interleaved [P, col, (negm|sum)] in t-major
        # column order so later blocks occupy later columns; the exact
        # two-level LSE combine happens on the host (removes Ln/table-load
        # and all small fixup ops from the tail).  The diagonal dot products
        # are also computed host-side from the same quantized fp8 inputs,
        # freeing PSUM bank 8 and the GpSimd/ones path.
        stats_all = inp.tile([P, 2 * 2 * MT * NB], F32, tag="stats_all")

        # ---- main two passes ----
        HC = MT * NB  # stats columns per pass
        for L in range(2):
            lhs = a_sh if L == 0 else b_sh
            rhs_t = b_f if L == 0 else a_f  # noqa
            edges = B_EDGES if L == 0 else A_EDGES
            # t outer / m inner: during the DMA ramp all MT psum tiles of a
            # given t consume the SAME 1024-wide rhs slice, so the PE extracts
            # 4x more work per DMA'd byte and never outruns HBM.
            for t in range(NB):
                for m in range(MT):
                    scol = 2 * (L * HC + t * MT + m)
                    ps = psum.tile([P, BLK], F32, tag="ps")
                    for j in range(BLK // SUB):
                        n0 = t * BLK + j * SUB
                        chi, off = chunk_of(edges, n0)
                        for s in range(KS):
                            nc.tensor.matmul(
                                ps[:, j * SUB : (j + 1) * SUB],
                                lhsT=lhs[:, 2 * s : 2 * s + 2, m * P : (m + 1) * P],
                                rhs=rhs_t[chi][:, 2 * s : 2 * s + 2, off : off + SUB],
                                start=(s == 0),
                                stop=(s == KS - 1),
                                perf_mode=PM_DR,
                            )
                    # block stats straight from PSUM
                    nc.vector.reduce_max(
                        out=stats_all[:, scol : scol + 1],
                        in_=ps,
                        axis=AX.X,
                        negate=True,
                    )
                    sc = scr.tile([P, BLK], DT_SCR, tag="escr")
                    nc.scalar.activation(
                        sc,
                        ps,
                        ACTF.Exp,
                        bias=stats_all[:, scol : scol + 1],
                        accum_out=stats_all[:, scol + 1 : scol + 2],
                    )
                # stream completed stats out eagerly so the final (blocking)
                # descriptor in the tail only covers the last few blocks
                if (L, t) == (0, NB - 1):
                    nc.gpsimd.dma_start(
                        out=stats_out[:, : 2 * HC], in_=stats_all[:, : 2 * HC]
                    )
                elif (L, t) == (1, NB - 2):
                    c0, c1 = 2 * HC, 2 * (2 * HC - MT)
                    nc.gpsimd.dma_start(
                        out=stats_out[:, c0:c1], in_=stats_all[:, c0:c1]
                    )
                elif (L, t) == (1, NB - 1):
                    c0 = 2 * (2 * HC - MT)
                    nc.gpsimd.dma_start(
                        out=stats_out[:, c0:], in_=stats_all[:, c0:]
                    )

    nc.compile()
    return nc


def _get_program(dt_in):
    key = str(dt_in)
    if key not in _prog_cache:
        _prog_cache[key] = _build_program(dt_in)
    return _prog_cache[key]


def kernel(out_ftir, out_raman, labels=None, log_tau=None, **_unused):
    global LAST_RESULTS
    out_ftir = np.asarray(out_ftir, dtype=np.float32)
    out_raman = np.asarray(out_raman, dtype=np.float32)
    tau = float(np.minimum(np.exp(np.float64(np.asarray(log_tau))), 100.0))

    np_dt = mybir.dt.np(DT_IN)
    aT = np.ascontiguousarray(
        np.clip((out_ftir * np.float32(tau)).T, -240.0, 240.0)
    ).astype(np_dt)
    bT = np.ascontiguousarray(np.clip(out_raman.T, -240.0, 240.0)).astype(np_dt)

    # diagonal logits host-side from the same quantized values the device
    # matmuls consume: diag[i] = sum_d aT[d,i] * bT[d,i]
    diag_full = np.einsum(
        "di,di->i",
        aT.astype(np.float32),
        bT.astype(np.float32),
        dtype=np.float64,
    )

    def pack_sh(xT):
        # [D, n] -> [p, (s h) e]: row d = s*256 + h*128 + p
        n = xT.shape[1]
        return np.ascontiguousarray(
            xT.reshape(KS, 2, P, n).transpose(2, 0, 1, 3).reshape(P, 2 * KS * n)
        )

    def pack_chunked(xT, edges):
        # [D, B] -> [p, ch (s h) e_chunk]
        x4 = xT.reshape(KS, 2, P, B)
        parts = [
            x4[:, :, :, edges[ch] : edges[ch + 1]].transpose(2, 0, 1, 3).reshape(P, -1)
            for ch in range(len(edges) - 1)
        ]
        return np.ascontiguousarray(np.concatenate(parts, axis=1))

    B_EDGES = [0, 1024, 2048, 3072, 4096]
    A_EDGES = [0, 2048, 4096]
    atf_h = pack_chunked(aT, A_EDGES)
    btf_h = pack_chunked(bT, B_EDGES)

    in_maps = []
    for c in range(NCORES):
        sl = slice(c * SH, (c + 1) * SH)
        in_maps.append(
            {
                "ats": pack_sh(aT[:, sl]),
                "bts": pack_sh(bT[:, sl]),
                "atf": atf_h,
                "btf": btf_h,
            }
        )

    nc = _get_program(DT_IN)
    res = run_bass_kernel_spmd(
        nc, in_maps, core_ids=list(range(NCORES)), trace=PROFILE
    )
    LAST_RESULTS = res

    s_lse = 0.0
    for r in res.results:
        # exact two-level LSE combine (float64):
        # LSE = M + log(sum_b s_b * exp(m_b - M)),  m_b = -negm
        # stats layout: [P, L, t, m, (negm|sum)], block index t combines
        st = r["stats"].astype(np.float64).reshape(P, 2, NB, MT, 2)
        mb = -st[..., 0]
        sb = st[..., 1]
        M = mb.max(axis=2, keepdims=True)
        lse = M[:, :, 0, :] + np.log((sb * np.exp(mb - M)).sum(axis=2))
        s_lse += float(lse.sum())
    s_diag = float(diag_full.sum())
    loss = (s_lse - 2.0 * s_diag) / (2.0 * B)
    return np.array(loss, dtype=np.float32)
